# revision 24
# baseline (speedup 1.0000x reference)
"""Trainium2 Bass kernel for nn_LinearRNN (B=16, T=4096, D_in=256, H=512, D_out=256).

  xp = x @ W_in.T                       [B, T, H]
  h_t = xp_t + h_{t-1} @ W_h.T          (W_h is diagonal -> elementwise scan)
  out = hs @ W_out.T                    [B, T, D_out]

Strategy: batch data-parallel over 8 cores (2 batch rows per core). Per core:
  - host pre-transposes x to [b, d, t] so the contraction dim lands on SBUF
    partitions; weights pre-transposed likewise.
  - matmul1 on TensorE produces xp tiles [h=128, t=512] in PSUM,
  - VectorE tensor_tensor_scan runs the recurrence along the free (t) axis
    with the per-h decay broadcast from a [128,1] column, carry chained
    across t-chunks via the previous tile's last column,
  - matmul2 on TensorE contracts h back to d_out, ScalarE copies PSUM->SBUF,
  - output [b, o, t] DMAs back and the host transposes to [b, t, o].
"""
from contextlib import ExitStack

import numpy as np

import concourse.bass as bass
import concourse.mybir as mybir
import concourse.tile as tile
from concourse import bacc
from concourse.bass_utils import run_bass_kernel_spmd

B, T, D_IN, HID, D_OUT = 16, 4096, 256, 512, 256
NCORES = 8
BPC = B // NCORES          # batch rows per core
TC = 512                   # t-chunk (PSUM bank = 512 fp32)
NCH = T // TC
ND = D_IN // 128           # 2  d-blocks
NH = HID // 128            # 4  h-blocks
NO = D_OUT // 128          # 2  o-blocks
OUT_HALF = T // 2

# 'f32'  : exact fp32 matmuls (4 cyc/row on PE)
# 'f32r' : fp32 storage, PE runs reduced-precision single-pass (1 cyc/row)
# 'bf16' : x/weights/hs cast to bf16 (halves input DMA, fastest PE)
MODE_DEFAULT = "f32r"

# schedule/tuning knobs (read by _build; cache key includes them)
CFG = dict(interleave=False, xp_bufs=5, op_bufs=3, hs_bufs=16,
           x_piece=512, out_piece=512)

_cache: dict = {}


def _build(mode: str) -> bass.Bass:
    f32 = mybir.dt.float32
    # f32r (tf32): the BIR verifier requires every producer of an fp32r
    # matmul operand to emit fp32r, DMAs included — so the input DRAM params
    # and SBUF tiles carry dt.float32r end-to-end (numpy repr is float32),
    # and the scan writes hs rounded to fp32r.
    dt_in = {"bf16": mybir.dt.bfloat16, "f32r": mybir.dt.float32r}.get(mode, f32)
    dt_hs = dt_in

    def mm(ap):
        return ap

    nc = bacc.Bacc(None, target_bir_lowering=False)

    xT = nc.declare_dram_parameter("xT", [BPC, D_IN, T], dt_in, isOutput=False)
    w_inT = nc.declare_dram_parameter("w_inT", [D_IN, HID], dt_in, isOutput=False)
    w_outT = nc.declare_dram_parameter("w_outT", [HID, D_OUT], dt_in, isOutput=False)
    dcols = nc.declare_dram_parameter("dcols", [128, NH], f32, isOutput=False)
    out = nc.declare_dram_parameter("out", [BPC, D_OUT, T], f32, isOutput=True)

    with tile.TileContext(nc) as tc, ExitStack() as ctx:
        const_pool = ctx.enter_context(tc.tile_pool(name="const", bufs=1))
        x_pool = ctx.enter_context(tc.tile_pool(name="xt", bufs=BPC * ND))
        o_pool = ctx.enter_context(tc.tile_pool(name="ot", bufs=8))
        hs_pool = ctx.enter_context(tc.tile_pool(name="hs", bufs=CFG["hs_bufs"]))
        xp_psum = ctx.enter_context(
            tc.tile_pool(name="xp", bufs=CFG["xp_bufs"], space=bass.MemorySpace.PSUM))
        op_psum = ctx.enter_context(
            tc.tile_pool(name="op", bufs=CFG["op_bufs"], space=bass.MemorySpace.PSUM))

        # DMA emission order is dispatch order per queue: first the matmul1
        # weights, then the first x pieces of batch 0 (unblocks PE ~4 us in),
        # then the remaining constants and the rest of x.
        XP_LEN = CFG["x_piece"]
        xt = {}
        for b in range(BPC):
            for dblk in range(ND):
                xt[(b, dblk)] = x_pool.tile([128, T], dt_in, name="xt", tag="xt")

        def load_x(b, dblk, piece):
            psl = slice(piece * XP_LEN, (piece + 1) * XP_LEN)
            nc.sync.dma_start(xt[(b, dblk)][:, psl],
                              xT[b, dblk * 128:(dblk + 1) * 128, psl])

        # first x pieces on the SP ring; all weights on the ACT ring so the
        # two HWDGE rings stream in parallel and PE unblocks ASAP
        for dblk in range(ND):
            load_x(0, dblk, 0)
        wi = []
        for dblk in range(ND):
            w = const_pool.tile([128, HID], dt_in, tag=f"wi{dblk}")
            nc.sync.dma_start(w[:], w_inT[dblk * 128:(dblk + 1) * 128, :])
            wi.append(w)
        wo = []
        for hblk in range(NH):
            w = const_pool.tile([128, D_OUT], dt_in, tag=f"wo{hblk}")
            nc.sync.dma_start(w[:], w_outT[hblk * 128:(hblk + 1) * 128, :])
            wo.append(w)
        dc = const_pool.tile([128, NH], f32, tag="dc")
        nc.sync.dma_start(dc[:], dcols[:])
        for piece in range(1, T // XP_LEN):
            for dblk in range(ND):
                load_x(0, dblk, piece)
        for b in range(1, BPC):
            for piece in range(T // XP_LEN):
                for dblk in range(ND):
                    load_x(b, dblk, piece)

        OP = CFG["out_piece"]
        ot = {}  # (b, oblk) -> current staging piece, created lazily

        prev_hs = {}

        def stage1(b, ic):
            """matmul1 + scan for one (batch, chunk): 4 h-block units."""
            tsl = slice(ic * TC, (ic + 1) * TC)
            for hblk in range(NH):
                xp = xp_psum.tile([128, TC], f32, name="xp", tag="xp")
                for dblk in range(ND):
                    nc.tensor.matmul(
                        xp[:],
                        mm(wi[dblk][:, hblk * 128:(hblk + 1) * 128]),
                        mm(xt[(b, dblk)][:, tsl]),
                        start=(dblk == 0), stop=(dblk == ND - 1))
                hs = hs_pool.tile([128, TC], dt_hs, name="hs", tag="hs")
                init = (0.0 if ic == 0 else prev_hs[(b, hblk)][:, TC - 1:TC])
                nc.vector.tensor_tensor_scan(
                    hs[:], dc[:, hblk:hblk + 1].to_broadcast((128, TC)),
                    xp[:], init,
                    op0=mybir.AluOpType.mult, op1=mybir.AluOpType.add)
                prev_hs[(b, hblk)] = hs

        def stage2(b, ic):
            """matmul2 + PSUM->SBUF copy (+ out DMA) for one (batch, chunk)."""
            q, csl = divmod(ic * TC, OP)
            for oblk in range(NO):
                op = op_psum.tile([128, TC], f32, name="op", tag="op")
                for hblk in range(NH):
                    nc.tensor.matmul(
                        op[:],
                        mm(wo[hblk][:, oblk * 128:(oblk + 1) * 128]),
                        mm(prev_hs[(b, hblk)][:]),
                        start=(hblk == 0), stop=(hblk == NH - 1))
                if csl == 0:
                    ot[(b, oblk)] = o_pool.tile([128, OP], f32,
                                                name="ot", tag="ot")
                last = (b == BPC - 1) and (ic == NCH - 1)
                nc.scalar.copy(ot[(b, oblk)][:, csl:csl + TC], op[:])
                if csl + TC == OP:
                    osl = out[b, oblk * 128:(oblk + 1) * 128,
                              q * OP:(q + 1) * OP]
                    if last:
                        # halve the final DMAs so they overlap the copies
                        h = OP // 2
                        nc.sync.dma_start(osl[:, :h], ot[(b, oblk)][:, :h])
                        nc.sync.dma_start(osl[:, h:], ot[(b, oblk)][:, h:])
                    else:
                        nc.sync.dma_start(osl, ot[(b, oblk)][:])

        if CFG["interleave"]:
            for ic in range(NCH):
                for b in range(BPC):
                    stage1(b, ic)
                for b in range(BPC):
                    stage2(b, ic)
        else:
            for b in range(BPC):
                for ic in range(NCH):
                    stage1(b, ic)
                    stage2(b, ic)

    nc.compile()
    return nc


def _prep_inputs(x, W_in, W_h, W_out, mode: str):
    npdt = np.float32
    if mode == "bf16":
        import ml_dtypes
        npdt = ml_dtypes.bfloat16
    xT = np.ascontiguousarray(np.transpose(np.asarray(x, np.float32), (0, 2, 1))).astype(npdt)
    w_inT = np.ascontiguousarray(np.asarray(W_in, np.float32).T).astype(npdt)
    w_outT = np.ascontiguousarray(np.asarray(W_out, np.float32).T).astype(npdt)
    d = np.ascontiguousarray(np.diagonal(np.asarray(W_h, np.float32)))
    dcols = np.ascontiguousarray(d.reshape(NH, 128).T, dtype=np.float32)
    in_maps = []
    for c in range(NCORES):
        in_maps.append({
            "xT": np.ascontiguousarray(xT[c * BPC:(c + 1) * BPC]),
            "w_inT": w_inT,
            "w_outT": w_outT,
            "dcols": dcols,
        })
    return in_maps


def _get_nc(mode: str = MODE_DEFAULT):
    key = (mode, tuple(sorted(CFG.items())))
    if key not in _cache:
        _cache[key] = _build(mode)
    return _cache[key]


def _run(x, W_in, W_h, W_out, mode: str = MODE_DEFAULT, **spmd_kwargs):
    nc = _get_nc(mode)
    in_maps = _prep_inputs(x, W_in, W_h, W_out, mode)
    res = run_bass_kernel_spmd(nc, in_maps, list(range(NCORES)), **spmd_kwargs)
    parts = [np.transpose(np.asarray(res.results[c]["out"]), (0, 2, 1))
             for c in range(NCORES)]
    full = np.concatenate(parts, axis=0).astype(np.float32)
    return full, res


def kernel(x, W_in, W_h, W_out):
    out, _ = _run(x, W_in, W_h, W_out)
    return out
